# revision 30
# baseline (speedup 1.0000x reference)
"""Annular patch embedding on 8 TRN2 NeuronCores.

Math: tokens[b, r, d] = sum_p x[b, p] * mask[r, p] * W[d, p]; out = tokens @
fc_w.T + fc_b. The rings are disjoint, so this is a segmented matmul over only
the ~39.4K pixels covered by rings. The fc projection is folded into the conv
weights on the host: V[o, p] = sum_d fc_w[o, d] * W[d, p], so the device
computes out[b, r, o] = sum_{p in ring r} x[b, p] * V[o, p] (+ bias via a
synthetic pixel with x == 1 and V column == fc_b).

Distribution: ring-sorted pixels are packed into 128-pixel contraction tiles,
40 tiles per core (8 cores x 40 = 320 slots for the 316 real tiles). Each core
runs the same SPMD graph: 5 PSUM accumulation groups with fixed tile counts
(19, 9, 6, 4, 2); a ring occupies an exact set of (core, group) slots, and the
host sums the per-slot partial outputs. The packing below covers every ring's
tile count exactly, so there is no zero-padding waste beyond the partial last
tile of each ring. No collectives are needed: every input byte is read by
exactly one core and the cross-piece reduction is a cheap host-side add.

The device graph is hand-scheduled raw Bass (no TileContext). Per core: input
chunks (x and V columns fused in consumption order) stream over BOTH HWDGE
rings (Sync + Scalar, one ring alone caps at ~260 GB/s; two hit the ~420 GB/s
limit); bf16 matmuls chase the chunk stream into per-group PSUM banks; DVE
casts psum to f16 and two output DMAs overlap the last accumulation group.
A group's completion semaphore is raised by the NEXT group's first matmul
(or a PE drain for the final group) so the DVE never reads a PSUM bank the
PE is still draining into. Dummy warm-up matmuls keep the PE busy >3.4us so
the HAM clock gate lifts to 2.4 GHz before real work. Nothing waits on the
output DMAs' ~2.5us write receipt: once queued, the SDMA hardware drains
them unconditionally, and the ~7us runtime epilogue that follows provides
ample margin before NEFF completion; their completion sems are write-only
and live outside the cleared range so the NEFF stays re-executable.
"""

import numpy as np
import ml_dtypes

import concourse.bass as bass
import concourse.mybir as mybir
import concourse.tile as tile
from concourse import bacc
from concourse.bass_utils import run_bass_kernel_spmd

IMG = 224
NPIX = IMG * IMG
B = 64
TOKEN_DIM = 256
OUT_DIM = 192
NUM_RINGS = 16
N_CORES = 8
P = 128

# PSUM accumulation groups per core (tiles per group); identical on all cores.
# Ordered big-to-small so the final group (last tiles to arrive) has minimal
# work between the last input chunk landing and the final output DMA.
GROUP_SIZES = (19, 9, 6, 4, 2)
T_CORE = sum(GROUP_SIZES)  # 40 tiles of 128 pixels per core
N_GROUPS = len(GROUP_SIZES)

# Ring r (tile counts 2,4,6,9,11,14,16,19,21,23,26,28,31,33,35,38) is split
# into pieces whose sizes are drawn from the per-core group sizes. Each piece
# occupies one (core, group) slot. Slot budget: 8 of each size; this table
# uses 6/8/8/8/8 of sizes 2/4/6/9/19 — an exact cover.
RING_DECOMP = (
    (2,), (4,), (6,), (9,),
    (2, 9), (2, 4, 4, 4), (2, 4, 4, 6), (19,),
    (2, 19), (4, 19), (2, 6, 9, 9), (9, 19),
    (6, 6, 19), (6, 9, 9, 9), (4, 6, 6, 19), (19, 19),
)

COMPUTE_DTYPE = "bf16"  # "bf16" or "f32"
MODE = "raw"  # "raw" (hand-scheduled Block) or "tile" (TileContext)
# Input tiles (x columns + V columns interleaved per chunk) are DMA'd in these
# chunk sizes, pipelined against the matmul stream: small first chunk so
# matmuls start early, small last chunk for a short tail. Even chunks go on
# the Sync HWDGE ring, odd chunks on the Scalar ring — one ring tops out at
# ~260 GB/s, two run at the ~420 GB/s fabric/HBM limit.
CHUNK_TILES = (4, 6, 8, 8, 6, 6, 2)
WARMUP_MMS = 32  # dummy matmuls to lift the PE HAM clock gate during DMA-in
OUT_DT = "f16"  # output staging dtype: "f16" halves the out DMA, err ~5e-4
TILE_COLS = B + OUT_DIM  # 256 fused columns per tile (64 x + 192 V)

# test.py hooks: extra kwargs for run_bass_kernel_spmd (e.g. trace=True), and
# the last BassKernelResults for timing introspection.
_RUN_KWARGS = {}
LAST_RESULTS = None

_GRAPH_CACHE = {}


def _chunk_bounds():
    """(t0, t1) tile ranges per DMA chunk."""
    assert sum(CHUNK_TILES) == T_CORE
    bounds, t = [], 0
    for ch in CHUNK_TILES:
        bounds.append((t, t + ch))
        t += ch
    return bounds


def _sb_offsets():
    """Per-tile column offsets of the x block and V block in the fused
    [128, T_CORE * TILE_COLS] layout: chunk c holds its tiles' x columns
    first, then its tiles' V columns, so DMA arrival order == use order."""
    xoff, voff = [0] * T_CORE, [0] * T_CORE
    for t0, t1 in _chunk_bounds():
        base = t0 * TILE_COLS
        for t in range(t0, t1):
            xoff[t] = base + (t - t0) * B
            voff[t] = base + (t1 - t0) * B + (t - t0) * OUT_DIM
    return xoff, voff


def _build_graph_raw(dt):
    out_dt = mybir.dt.float16 if OUT_DT == "f16" else mybir.dt.float32
    nc = bass.Bass("TRN2", debug=False, num_devices=N_CORES)
    data = nc.declare_dram_parameter(
        "data", [P, T_CORE * TILE_COLS], dt, isOutput=False
    )
    out = nc.declare_dram_parameter(
        "out", [B, N_GROUPS * OUT_DIM], out_dt, isOutput=True
    )

    data_sb = nc.alloc_sbuf_tensor("data_sb", [P, T_CORE * TILE_COLS], dt)
    out_sb = nc.alloc_sbuf_tensor("out_sb", [B, N_GROUPS * OUT_DIM], out_dt)
    warm_sb = nc.alloc_sbuf_tensor("warm_sb", [P, B + 128], dt)

    pss = [
        nc.alloc_psum_tensor(f"ps{g}", [B, OUT_DIM], mybir.dt.float32)
        for g in range(N_GROUPS)
    ]
    warm_ps = nc.alloc_psum_tensor("warm_ps", [B, 128], mybir.dt.float32)

    even_sem = nc.alloc_semaphore("even_sem")
    odd_sem = nc.alloc_semaphore("odd_sem")
    mm_sem = nc.alloc_semaphore("mm_sem")
    copy_sem = nc.alloc_semaphore("copy_sem")
    # Write-only completion sems for the output DMAs. Nothing waits on them
    # and they are deliberately NOT in the cleared range: their value is
    # meaningless across executions, so leaving them dirty is harmless.
    out_e_sem = nc.alloc_semaphore("out_e_sem")
    out_o_sem = nc.alloc_semaphore("out_o_sem")
    sem_nums = sorted(s.num for s in (even_sem, odd_sem, mm_sem, copy_sem))
    assert sem_nums == list(range(sem_nums[0], sem_nums[0] + 4))
    sem_range = range(sem_nums[0], sem_nums[-1] + 1)

    chunks = _chunk_bounds()
    xoff, voff = _sb_offsets()
    n_even = (len(chunks) + 1) // 2
    n_odd = len(chunks) // 2
    out1_cols = (N_GROUPS - 1) * OUT_DIM  # groups 0..3 in one DMA, group 4 last

    def _chunk_dma(eng, c):
        t0, t1 = chunks[c]
        eng.dma_start(
            data_sb[:, t0 * TILE_COLS : t1 * TILE_COLS],
            data[:, t0 * TILE_COLS : t1 * TILE_COLS],
        ).then_inc(even_sem if c % 2 == 0 else odd_sem, 16)

    # Issue the first chunk of each ring from the entry basic block, ahead of
    # the Block-entry branch, so the DMA pipeline starts as early as possible.
    _chunk_dma(nc.sync, 0)
    _chunk_dma(nc.scalar, 1)

    with nc.Block(no_gpsimd_drain=True) as block:

        @block.sync
        def _(sync):
            for c in range(2, len(chunks), 2):
                _chunk_dma(sync, c)
            sync.wait_ge(copy_sem, N_GROUPS)
            # No completion semaphore on the output DMAs: once queued on the
            # HWDGE ring the SDMA hardware drains it unconditionally, and the
            # ~7us runtime epilogue that follows dwarfs the ~0.7us flight.
            # Waiting for the write receipt (~2.5us) would serialize it into
            # the critical path for nothing.
            sync.dma_start(out[:, out1_cols:], out_sb[:, out1_cols:]).then_inc(
                out_e_sem, 16
            )

        @block.scalar
        def _(scalar):
            for c in range(3, len(chunks), 2):
                _chunk_dma(scalar, c)
            scalar.wait_ge(copy_sem, N_GROUPS - 1)
            scalar.dma_start(out[:, :out1_cols], out_sb[:, :out1_cols]).then_inc(
                out_o_sem, 16
            )

        @block.tensor
        def _(tensor):
            # Dummy matmuls (garbage data, dead psum bank) to keep the PE
            # busy while inputs stream in, so real matmuls run at 2.4 GHz.
            for _ in range(WARMUP_MMS):
                tensor.matmul(
                    warm_ps[:], warm_sb[:, :B], warm_sb[:, B:], start=True, stop=True
                )
            t = 0
            chunk = -1
            pending_inc = 0  # groups whose psum is complete once a later MM runs
            for g, gsz in enumerate(GROUP_SIZES):
                for i in range(gsz):
                    while chunk < len(chunks) - 1 and t >= chunks[chunk + 1][0]:
                        chunk += 1
                        sem = even_sem if chunk % 2 == 0 else odd_sem
                        tensor.wait_ge(sem, 16 * (chunk // 2 + 1))
                    mm = tensor.matmul(
                        pss[g][:],
                        data_sb[:, xoff[t] : xoff[t] + B],
                        data_sb[:, voff[t] : voff[t] + OUT_DIM],
                        start=(i == 0),
                        stop=(i == gsz - 1),
                    )
                    # Signal group g-1 complete from group g's FIRST matmul:
                    # by the time this matmul retires, the previous group's
                    # last psum writes have fully drained through the PE pipe
                    # (in-order array). Inc'ing on a group's own last matmul
                    # can fire before its drain lands -> PSUM collision when
                    # the DVE copy reads that bank.
                    if i == 0 and pending_inc:
                        mm.then_inc(mm_sem, pending_inc)
                        pending_inc = 0
                    t += 1
                pending_inc += 1
            # Final group(s): a PE drain completes only once all psum writes
            # have landed.
            tensor.drain().then_inc(mm_sem, pending_inc)

        @block.vector
        def _(vector):
            for g in range(N_GROUPS):
                vector.wait_ge(mm_sem, g + 1)
                vector.tensor_copy(
                    out_sb[:, g * OUT_DIM : (g + 1) * OUT_DIM], pss[g][:]
                ).then_inc(copy_sem, 1)

    # After the block's end-of-kernel barrier (which now implies both output
    # DMAs have landed): restore semaphores to zero so the NEFF can be
    # re-executed without a reload.
    nc.gpsimd.sem_clear(sem_range)
    return nc


def _build_graph_tile(dt):
    dma_chunk = 5
    nc = bacc.Bacc("TRN2", target_bir_lowering=False, debug=False, num_devices=N_CORES)
    xs = nc.declare_dram_parameter("xs", [P, T_CORE * B], dt, isOutput=False)
    vs = nc.declare_dram_parameter("vs", [P, T_CORE * OUT_DIM], dt, isOutput=False)
    out = nc.declare_dram_parameter(
        "out", [B, N_GROUPS * OUT_DIM], mybir.dt.float32, isOutput=True
    )

    with tile.TileContext(nc) as tc:
        with (
            tc.tile_pool(name="data", bufs=1) as data,
            tc.tile_pool(name="psum", bufs=N_GROUPS, space="PSUM") as psum_pool,
        ):
            nchunks = -(-T_CORE // dma_chunk)
            xs_sb, vs_sb = [None] * T_CORE, [None] * T_CORE
            for c in range(nchunks):
                t0, t1 = c * dma_chunk, min((c + 1) * dma_chunk, T_CORE)
                xt = data.tile([P, (t1 - t0) * B], dt, tag=f"xs{c}")
                nc.sync.dma_start(xt[:], xs[:, t0 * B : t1 * B])
                vt = data.tile([P, (t1 - t0) * OUT_DIM], dt, tag=f"vs{c}")
                nc.sync.dma_start(vt[:], vs[:, t0 * OUT_DIM : t1 * OUT_DIM])
                for t in range(t0, t1):
                    xs_sb[t] = (xt, t - t0)
                    vs_sb[t] = (vt, t - t0)

            out_sb = data.tile([B, N_GROUPS * OUT_DIM], mybir.dt.float32, tag="out")
            t = 0
            for g, gsz in enumerate(GROUP_SIZES):
                ps = psum_pool.tile([B, OUT_DIM], mybir.dt.float32, tag="ps")
                for i in range(gsz):
                    xt, xo = xs_sb[t]
                    vt, vo = vs_sb[t]
                    nc.tensor.matmul(
                        ps[:],
                        xt[:, xo * B : (xo + 1) * B],
                        vt[:, vo * OUT_DIM : (vo + 1) * OUT_DIM],
                        start=(i == 0),
                        stop=(i == gsz - 1),
                    )
                    t += 1
                nc.vector.tensor_copy(out_sb[:, g * OUT_DIM : (g + 1) * OUT_DIM], ps[:])
            nc.sync.dma_start(out[:], out_sb[:])

    nc.compile()
    return nc


def _get_graph(dt):
    key = (MODE, dt)
    if key not in _GRAPH_CACHE:
        build = _build_graph_raw if MODE == "raw" else _build_graph_tile
        _GRAPH_CACHE[key] = build(dt)
    return _GRAPH_CACHE[key]


def _layout(masks):
    """Ring id per pixel and the ring-piece -> (core, group) slot assignment."""
    m = np.asarray(masks, dtype=np.float32).reshape(NUM_RINGS, NPIX) > 0.5
    ring = np.where(m.any(axis=0), m.argmax(axis=0), -1)

    offs = np.concatenate(([0], np.cumsum(GROUP_SIZES)))
    free = {}
    for core in range(N_CORES):
        for g, sz in enumerate(GROUP_SIZES):
            free.setdefault(sz, []).append((core, g, int(offs[g])))

    pieces = []  # (ring, core, group, core_tile_off, ring_tile_off, size)
    for r in range(NUM_RINGS):
        cnt = int((ring == r).sum())
        tiles = -(-cnt // P)
        decomp = RING_DECOMP[r]
        assert sum(decomp) == tiles, (r, cnt, tiles, decomp)
        assert cnt < tiles * P, f"ring {r} has no pad slot for the bias"
        roff = 0
        for sz in decomp:
            core, g, toff = free[sz].pop(0)
            pieces.append((r, core, g, toff, roff, sz))
            roff += sz
    return ring, pieces


def kernel(x, tokens_weights, fc_w, fc_b, masks):
    x = np.asarray(x, dtype=np.float32).reshape(B, NPIX)
    W = np.asarray(tokens_weights, dtype=np.float32).reshape(TOKEN_DIM, NPIX)
    fc_w = np.asarray(fc_w, dtype=np.float32)
    fc_b = np.asarray(fc_b, dtype=np.float32)

    # Fold the 256->192 fc into the conv weights: V[o, p] = fc_w @ W.
    V = (fc_w.astype(np.float64) @ W.astype(np.float64)).astype(np.float32)

    ring, pieces = _layout(masks)

    # Gather index per (core, tile slot, lane): pixel id, -1 pad, -2 bias.
    gidx = np.full((N_CORES, T_CORE * P), -1, dtype=np.int64)
    for r in range(NUM_RINGS):
        pix = np.nonzero(ring == r)[0]
        tiles = -(-len(pix) // P)
        arr = np.full(tiles * P, -1, dtype=np.int64)
        arr[: len(pix)] = pix
        arr[len(pix)] = -2  # bias slot (exactly one per ring)
        for rr, core, g, toff, roff, sz in pieces:
            if rr == r:
                gidx[core, toff * P : (toff + sz) * P] = arr[roff * P : (roff + sz) * P]

    sel = (gidx >= 0)[..., None]
    cl = np.clip(gidx, 0, None)
    xs_full = np.where(sel, x.T[cl], np.float32(0))  # [cores, T*P, B]
    xs_full[gidx == -2] = 1.0
    vs_full = np.where(sel, V.T[cl], np.float32(0))  # [cores, T*P, OUT_DIM]
    vs_full[gidx == -2] = fc_b

    dt_np = ml_dtypes.bfloat16 if COMPUTE_DTYPE == "bf16" else np.float32
    xs_dev = (
        xs_full.reshape(N_CORES, T_CORE, P, B).transpose(0, 2, 1, 3)
        .reshape(N_CORES, P, T_CORE * B).astype(dt_np)
    )
    vs_dev = (
        vs_full.reshape(N_CORES, T_CORE, P, OUT_DIM).transpose(0, 2, 1, 3)
        .reshape(N_CORES, P, T_CORE * OUT_DIM).astype(dt_np)
    )
    if MODE == "raw":
        # Fused layout: per chunk, the x columns of its tiles then the V
        # columns of its tiles — matches _sb_offsets on the device.
        data_dev = np.empty((N_CORES, P, T_CORE * TILE_COLS), dtype=dt_np)
        for t0, t1 in _chunk_bounds():
            base = t0 * TILE_COLS
            xw = (t1 - t0) * B
            data_dev[:, :, base : base + xw] = xs_dev[:, :, t0 * B : t1 * B]
            data_dev[:, :, base + xw : t1 * TILE_COLS] = vs_dev[
                :, :, t0 * OUT_DIM : t1 * OUT_DIM
            ]
        in_maps = [{"data": np.ascontiguousarray(data_dev[c])} for c in range(N_CORES)]
    else:
        in_maps = [
            {
                "xs": np.ascontiguousarray(xs_dev[c]),
                "vs": np.ascontiguousarray(vs_dev[c]),
            }
            for c in range(N_CORES)
        ]

    nc = _get_graph(mybir.dt.from_np(np.dtype(dt_np)))
    res = run_bass_kernel_spmd(
        nc, in_maps, core_ids=list(range(N_CORES)), **_RUN_KWARGS
    )
    global LAST_RESULTS
    LAST_RESULTS = res

    out = np.zeros((B, NUM_RINGS, OUT_DIM), dtype=np.float32)
    for r, core, g, toff, roff, sz in pieces:
        part = res.results[core]["out"][:, g * OUT_DIM : (g + 1) * OUT_DIM]
        out[:, r, :] += part.astype(np.float32)
    return out


# revision 31
# speedup vs baseline: 1.0129x; 1.0129x over previous
"""Annular patch embedding on 8 TRN2 NeuronCores.

Math: tokens[b, r, d] = sum_p x[b, p] * mask[r, p] * W[d, p]; out = tokens @
fc_w.T + fc_b. The rings are disjoint, so this is a segmented matmul over only
the ~39.4K pixels covered by rings. The fc projection is folded into the conv
weights on the host: V[o, p] = sum_d fc_w[o, d] * W[d, p], so the device
computes out[b, r, o] = sum_{p in ring r} x[b, p] * V[o, p] (+ bias via a
synthetic pixel with x == 1 and V column == fc_b).

Distribution: ring-sorted pixels are packed into 128-pixel contraction tiles,
40 tiles per core (8 cores x 40 = 320 slots for the 316 real tiles). Each core
runs the same SPMD graph: 5 PSUM accumulation groups with fixed tile counts
(19, 9, 6, 4, 2); a ring occupies an exact set of (core, group) slots, and the
host sums the per-slot partial outputs. The packing below covers every ring's
tile count exactly, so there is no zero-padding waste beyond the partial last
tile of each ring. No collectives are needed: every input byte is read by
exactly one core and the cross-piece reduction is a cheap host-side add.

The device graph is hand-scheduled raw Bass (no TileContext). Per core: input
chunks (x and V columns fused in consumption order) stream over BOTH HWDGE
rings (Sync + Scalar, one ring alone caps at ~260 GB/s; two hit the ~420 GB/s
limit); bf16 matmuls chase the chunk stream into per-group PSUM banks; DVE
casts psum to f16 and two output DMAs overlap the last accumulation group.
A group's completion semaphore is raised by the NEXT group's first matmul
(or a PE drain for the final group) so the DVE never reads a PSUM bank the
PE is still draining into. Dummy warm-up matmuls keep the PE busy >3.4us so
the HAM clock gate lifts to 2.4 GHz before real work. Nothing waits on the
output DMAs' ~2.5us write receipt: once queued, the SDMA hardware drains
them unconditionally, and the ~7us runtime epilogue that follows provides
ample margin before NEFF completion; their completion sems are write-only
and live outside the cleared range so the NEFF stays re-executable.
"""

import numpy as np
import ml_dtypes

import concourse.bass as bass
import concourse.mybir as mybir
import concourse.tile as tile
from concourse import bacc
from concourse.bass_utils import run_bass_kernel_spmd

IMG = 224
NPIX = IMG * IMG
B = 64
TOKEN_DIM = 256
OUT_DIM = 192
NUM_RINGS = 16
N_CORES = 8
P = 128

# PSUM accumulation groups per core (tiles per group); identical on all cores.
# Ordered big-to-small so the final group (last tiles to arrive) has minimal
# work between the last input chunk landing and the final output DMA.
GROUP_SIZES = (19, 9, 6, 4, 2)
T_CORE = sum(GROUP_SIZES)  # 40 tiles of 128 pixels per core
N_GROUPS = len(GROUP_SIZES)

# Ring r (tile counts 2,4,6,9,11,14,16,19,21,23,26,28,31,33,35,38) is split
# into pieces whose sizes are drawn from the per-core group sizes. Each piece
# occupies one (core, group) slot. Slot budget: 8 of each size; this table
# uses 6/8/8/8/8 of sizes 2/4/6/9/19 — an exact cover.
RING_DECOMP = (
    (2,), (4,), (6,), (9,),
    (2, 9), (2, 4, 4, 4), (2, 4, 4, 6), (19,),
    (2, 19), (4, 19), (2, 6, 9, 9), (9, 19),
    (6, 6, 19), (6, 9, 9, 9), (4, 6, 6, 19), (19, 19),
)

COMPUTE_DTYPE = "f16"  # "f16", "bf16", or "f32": f16 is the same
# speed as bf16 (2 bytes, full-rate PE) but has 10 mantissa bits, cutting the
# quantization error ~8x. All values here are far inside f16 range.
MODE = "raw"  # "raw" (hand-scheduled Block) or "tile" (TileContext)
# Input tiles (x columns + V columns interleaved per chunk) are DMA'd in these
# chunk sizes, pipelined against the matmul stream: small first chunk so
# matmuls start early, small last chunk for a short tail. Even chunks go on
# the Sync HWDGE ring, odd chunks on the Scalar ring — one ring tops out at
# ~260 GB/s, two run at the ~420 GB/s fabric/HBM limit.
CHUNK_TILES = (4, 6, 8, 8, 6, 6, 2)
WARMUP_MMS = 32  # dummy matmuls to lift the PE HAM clock gate during DMA-in
OUT_DT = "f16"  # output staging dtype: "f16" halves the out DMA, err ~5e-4
TILE_COLS = B + OUT_DIM  # 256 fused columns per tile (64 x + 192 V)

# test.py hooks: extra kwargs for run_bass_kernel_spmd (e.g. trace=True), and
# the last BassKernelResults for timing introspection.
_RUN_KWARGS = {}
LAST_RESULTS = None

_GRAPH_CACHE = {}


def _chunk_bounds():
    """(t0, t1) tile ranges per DMA chunk."""
    assert sum(CHUNK_TILES) == T_CORE
    bounds, t = [], 0
    for ch in CHUNK_TILES:
        bounds.append((t, t + ch))
        t += ch
    return bounds


def _sb_offsets():
    """Per-tile column offsets of the x block and V block in the fused
    [128, T_CORE * TILE_COLS] layout: chunk c holds its tiles' x columns
    first, then its tiles' V columns, so DMA arrival order == use order."""
    xoff, voff = [0] * T_CORE, [0] * T_CORE
    for t0, t1 in _chunk_bounds():
        base = t0 * TILE_COLS
        for t in range(t0, t1):
            xoff[t] = base + (t - t0) * B
            voff[t] = base + (t1 - t0) * B + (t - t0) * OUT_DIM
    return xoff, voff


def _build_graph_raw(dt):
    out_dt = mybir.dt.float16 if OUT_DT == "f16" else mybir.dt.float32
    nc = bass.Bass("TRN2", debug=False, num_devices=N_CORES)
    data = nc.declare_dram_parameter(
        "data", [P, T_CORE * TILE_COLS], dt, isOutput=False
    )
    out = nc.declare_dram_parameter(
        "out", [B, N_GROUPS * OUT_DIM], out_dt, isOutput=True
    )

    data_sb = nc.alloc_sbuf_tensor("data_sb", [P, T_CORE * TILE_COLS], dt)
    out_sb = nc.alloc_sbuf_tensor("out_sb", [B, N_GROUPS * OUT_DIM], out_dt)
    warm_sb = nc.alloc_sbuf_tensor("warm_sb", [P, B + 128], dt)

    pss = [
        nc.alloc_psum_tensor(f"ps{g}", [B, OUT_DIM], mybir.dt.float32)
        for g in range(N_GROUPS)
    ]
    warm_ps = nc.alloc_psum_tensor("warm_ps", [B, 128], mybir.dt.float32)

    even_sem = nc.alloc_semaphore("even_sem")
    odd_sem = nc.alloc_semaphore("odd_sem")
    mm_sem = nc.alloc_semaphore("mm_sem")
    copy_sem = nc.alloc_semaphore("copy_sem")
    # Write-only completion sems for the output DMAs. Nothing waits on them
    # and they are deliberately NOT in the cleared range: their value is
    # meaningless across executions, so leaving them dirty is harmless.
    out_e_sem = nc.alloc_semaphore("out_e_sem")
    out_o_sem = nc.alloc_semaphore("out_o_sem")
    sem_nums = sorted(s.num for s in (even_sem, odd_sem, mm_sem, copy_sem))
    assert sem_nums == list(range(sem_nums[0], sem_nums[0] + 4))
    sem_range = range(sem_nums[0], sem_nums[-1] + 1)

    chunks = _chunk_bounds()
    xoff, voff = _sb_offsets()
    n_even = (len(chunks) + 1) // 2
    n_odd = len(chunks) // 2
    out1_cols = (N_GROUPS - 1) * OUT_DIM  # groups 0..3 in one DMA, group 4 last

    def _chunk_dma(eng, c):
        t0, t1 = chunks[c]
        eng.dma_start(
            data_sb[:, t0 * TILE_COLS : t1 * TILE_COLS],
            data[:, t0 * TILE_COLS : t1 * TILE_COLS],
        ).then_inc(even_sem if c % 2 == 0 else odd_sem, 16)

    # Issue the first chunk of each ring from the entry basic block, ahead of
    # the Block-entry branch, so the DMA pipeline starts as early as possible.
    _chunk_dma(nc.sync, 0)
    _chunk_dma(nc.scalar, 1)

    with nc.Block(no_gpsimd_drain=True) as block:

        @block.sync
        def _(sync):
            for c in range(2, len(chunks), 2):
                _chunk_dma(sync, c)
            sync.wait_ge(copy_sem, N_GROUPS)
            # No completion semaphore on the output DMAs: once queued on the
            # HWDGE ring the SDMA hardware drains it unconditionally, and the
            # ~7us runtime epilogue that follows dwarfs the ~0.7us flight.
            # Waiting for the write receipt (~2.5us) would serialize it into
            # the critical path for nothing.
            sync.dma_start(out[:, out1_cols:], out_sb[:, out1_cols:]).then_inc(
                out_e_sem, 16
            )

        @block.scalar
        def _(scalar):
            for c in range(3, len(chunks), 2):
                _chunk_dma(scalar, c)
            scalar.wait_ge(copy_sem, N_GROUPS - 1)
            scalar.dma_start(out[:, :out1_cols], out_sb[:, :out1_cols]).then_inc(
                out_o_sem, 16
            )

        @block.tensor
        def _(tensor):
            # Dummy matmuls (garbage data, dead psum bank) to keep the PE
            # busy while inputs stream in, so real matmuls run at 2.4 GHz.
            for _ in range(WARMUP_MMS):
                tensor.matmul(
                    warm_ps[:], warm_sb[:, :B], warm_sb[:, B:], start=True, stop=True
                )
            t = 0
            chunk = -1
            pending_inc = 0  # groups whose psum is complete once a later MM runs
            for g, gsz in enumerate(GROUP_SIZES):
                for i in range(gsz):
                    while chunk < len(chunks) - 1 and t >= chunks[chunk + 1][0]:
                        chunk += 1
                        sem = even_sem if chunk % 2 == 0 else odd_sem
                        tensor.wait_ge(sem, 16 * (chunk // 2 + 1))
                    mm = tensor.matmul(
                        pss[g][:],
                        data_sb[:, xoff[t] : xoff[t] + B],
                        data_sb[:, voff[t] : voff[t] + OUT_DIM],
                        start=(i == 0),
                        stop=(i == gsz - 1),
                    )
                    # Signal group g-1 complete from group g's FIRST matmul:
                    # by the time this matmul retires, the previous group's
                    # last psum writes have fully drained through the PE pipe
                    # (in-order array). Inc'ing on a group's own last matmul
                    # can fire before its drain lands -> PSUM collision when
                    # the DVE copy reads that bank.
                    if i == 0 and pending_inc:
                        mm.then_inc(mm_sem, pending_inc)
                        pending_inc = 0
                    t += 1
                pending_inc += 1
            # Final group(s): a PE drain completes only once all psum writes
            # have landed.
            tensor.drain().then_inc(mm_sem, pending_inc)

        @block.vector
        def _(vector):
            for g in range(N_GROUPS):
                vector.wait_ge(mm_sem, g + 1)
                vector.tensor_copy(
                    out_sb[:, g * OUT_DIM : (g + 1) * OUT_DIM], pss[g][:]
                ).then_inc(copy_sem, 1)

    # After the block's end-of-kernel barrier (which now implies both output
    # DMAs have landed): restore semaphores to zero so the NEFF can be
    # re-executed without a reload.
    nc.gpsimd.sem_clear(sem_range)
    return nc


def _build_graph_tile(dt):
    dma_chunk = 5
    nc = bacc.Bacc("TRN2", target_bir_lowering=False, debug=False, num_devices=N_CORES)
    xs = nc.declare_dram_parameter("xs", [P, T_CORE * B], dt, isOutput=False)
    vs = nc.declare_dram_parameter("vs", [P, T_CORE * OUT_DIM], dt, isOutput=False)
    out = nc.declare_dram_parameter(
        "out", [B, N_GROUPS * OUT_DIM], mybir.dt.float32, isOutput=True
    )

    with tile.TileContext(nc) as tc:
        with (
            tc.tile_pool(name="data", bufs=1) as data,
            tc.tile_pool(name="psum", bufs=N_GROUPS, space="PSUM") as psum_pool,
        ):
            nchunks = -(-T_CORE // dma_chunk)
            xs_sb, vs_sb = [None] * T_CORE, [None] * T_CORE
            for c in range(nchunks):
                t0, t1 = c * dma_chunk, min((c + 1) * dma_chunk, T_CORE)
                xt = data.tile([P, (t1 - t0) * B], dt, tag=f"xs{c}")
                nc.sync.dma_start(xt[:], xs[:, t0 * B : t1 * B])
                vt = data.tile([P, (t1 - t0) * OUT_DIM], dt, tag=f"vs{c}")
                nc.sync.dma_start(vt[:], vs[:, t0 * OUT_DIM : t1 * OUT_DIM])
                for t in range(t0, t1):
                    xs_sb[t] = (xt, t - t0)
                    vs_sb[t] = (vt, t - t0)

            out_sb = data.tile([B, N_GROUPS * OUT_DIM], mybir.dt.float32, tag="out")
            t = 0
            for g, gsz in enumerate(GROUP_SIZES):
                ps = psum_pool.tile([B, OUT_DIM], mybir.dt.float32, tag="ps")
                for i in range(gsz):
                    xt, xo = xs_sb[t]
                    vt, vo = vs_sb[t]
                    nc.tensor.matmul(
                        ps[:],
                        xt[:, xo * B : (xo + 1) * B],
                        vt[:, vo * OUT_DIM : (vo + 1) * OUT_DIM],
                        start=(i == 0),
                        stop=(i == gsz - 1),
                    )
                    t += 1
                nc.vector.tensor_copy(out_sb[:, g * OUT_DIM : (g + 1) * OUT_DIM], ps[:])
            nc.sync.dma_start(out[:], out_sb[:])

    nc.compile()
    return nc


def _get_graph(dt):
    key = (MODE, dt)
    if key not in _GRAPH_CACHE:
        build = _build_graph_raw if MODE == "raw" else _build_graph_tile
        _GRAPH_CACHE[key] = build(dt)
    return _GRAPH_CACHE[key]


def _layout(masks):
    """Ring id per pixel and the ring-piece -> (core, group) slot assignment."""
    m = np.asarray(masks, dtype=np.float32).reshape(NUM_RINGS, NPIX) > 0.5
    ring = np.where(m.any(axis=0), m.argmax(axis=0), -1)

    offs = np.concatenate(([0], np.cumsum(GROUP_SIZES)))
    free = {}
    for core in range(N_CORES):
        for g, sz in enumerate(GROUP_SIZES):
            free.setdefault(sz, []).append((core, g, int(offs[g])))

    pieces = []  # (ring, core, group, core_tile_off, ring_tile_off, size)
    for r in range(NUM_RINGS):
        cnt = int((ring == r).sum())
        tiles = -(-cnt // P)
        decomp = RING_DECOMP[r]
        assert sum(decomp) == tiles, (r, cnt, tiles, decomp)
        assert cnt < tiles * P, f"ring {r} has no pad slot for the bias"
        roff = 0
        for sz in decomp:
            core, g, toff = free[sz].pop(0)
            pieces.append((r, core, g, toff, roff, sz))
            roff += sz
    return ring, pieces


def kernel(x, tokens_weights, fc_w, fc_b, masks):
    x = np.asarray(x, dtype=np.float32).reshape(B, NPIX)
    W = np.asarray(tokens_weights, dtype=np.float32).reshape(TOKEN_DIM, NPIX)
    fc_w = np.asarray(fc_w, dtype=np.float32)
    fc_b = np.asarray(fc_b, dtype=np.float32)

    # Fold the 256->192 fc into the conv weights: V[o, p] = fc_w @ W.
    V = (fc_w.astype(np.float64) @ W.astype(np.float64)).astype(np.float32)

    ring, pieces = _layout(masks)

    # Gather index per (core, tile slot, lane): pixel id, -1 pad, -2 bias.
    gidx = np.full((N_CORES, T_CORE * P), -1, dtype=np.int64)
    for r in range(NUM_RINGS):
        pix = np.nonzero(ring == r)[0]
        tiles = -(-len(pix) // P)
        arr = np.full(tiles * P, -1, dtype=np.int64)
        arr[: len(pix)] = pix
        arr[len(pix)] = -2  # bias slot (exactly one per ring)
        for rr, core, g, toff, roff, sz in pieces:
            if rr == r:
                gidx[core, toff * P : (toff + sz) * P] = arr[roff * P : (roff + sz) * P]

    sel = (gidx >= 0)[..., None]
    cl = np.clip(gidx, 0, None)
    xs_full = np.where(sel, x.T[cl], np.float32(0))  # [cores, T*P, B]
    xs_full[gidx == -2] = 1.0
    vs_full = np.where(sel, V.T[cl], np.float32(0))  # [cores, T*P, OUT_DIM]
    vs_full[gidx == -2] = fc_b

    dt_np = {
        "f16": np.float16, "bf16": ml_dtypes.bfloat16, "f32": np.float32
    }[COMPUTE_DTYPE]
    xs_dev = (
        xs_full.reshape(N_CORES, T_CORE, P, B).transpose(0, 2, 1, 3)
        .reshape(N_CORES, P, T_CORE * B).astype(dt_np)
    )
    vs_dev = (
        vs_full.reshape(N_CORES, T_CORE, P, OUT_DIM).transpose(0, 2, 1, 3)
        .reshape(N_CORES, P, T_CORE * OUT_DIM).astype(dt_np)
    )
    if MODE == "raw":
        # Fused layout: per chunk, the x columns of its tiles then the V
        # columns of its tiles — matches _sb_offsets on the device.
        data_dev = np.empty((N_CORES, P, T_CORE * TILE_COLS), dtype=dt_np)
        for t0, t1 in _chunk_bounds():
            base = t0 * TILE_COLS
            xw = (t1 - t0) * B
            data_dev[:, :, base : base + xw] = xs_dev[:, :, t0 * B : t1 * B]
            data_dev[:, :, base + xw : t1 * TILE_COLS] = vs_dev[
                :, :, t0 * OUT_DIM : t1 * OUT_DIM
            ]
        in_maps = [{"data": np.ascontiguousarray(data_dev[c])} for c in range(N_CORES)]
    else:
        in_maps = [
            {
                "xs": np.ascontiguousarray(xs_dev[c]),
                "vs": np.ascontiguousarray(vs_dev[c]),
            }
            for c in range(N_CORES)
        ]

    nc = _get_graph(mybir.dt.from_np(np.dtype(dt_np)))
    res = run_bass_kernel_spmd(
        nc, in_maps, core_ids=list(range(N_CORES)), **_RUN_KWARGS
    )
    global LAST_RESULTS
    LAST_RESULTS = res

    out = np.zeros((B, NUM_RINGS, OUT_DIM), dtype=np.float32)
    for r, core, g, toff, roff, sz in pieces:
        part = res.results[core]["out"][:, g * OUT_DIM : (g + 1) * OUT_DIM]
        out[:, r, :] += part.astype(np.float32)
    return out
